# revision 68
# baseline (speedup 1.0000x reference)
"""Multi-head self-attention block on Trainium2, 8-core SPMD.

Problem (fixed shapes): x(2,2048,1024), causal-additive mask(2,2048,2048),
w_qkv(1024,3072), b_qkv(3072), w_out(1024,1024), b_out(1024).
out = MHSA(x) with H=16 heads, head_dim=64.

Sharding strategy:
  - QKV projection + attention: tensor-parallel over heads (2 heads/core).
    Each core computes Q^T,K^T,V for its 2 heads over all 4096 tokens.
  - All matmul operands are bf16 (PSUM accumulation stays fp32): on HW,
    fp32 matmuls stream at ~1.2 ns/row vs ~0.5 ns/row for bf16.
  - Scores are computed transposed ([keys, q] layout) so softmax runs
    without any on-chip transposes: exp on ScalarE, the denominator comes
    from an extra all-ones column appended to V (one fused matmul), and
    causal masking zeroes exp(scores) above the diagonal with a GpSimd
    affine_select restricted to the 128-wide partial sub-block.
  - Diagonal k-chunks restrict scores/exp/AV to the live query window
    [128*m, 512), skipping the fully-masked left columns.
  - Softmax skips max-subtraction: scores for this problem are O(10) and
    exp() is computed in fp32, so normalization is exact enough.
  - 1/denominator via the DVE fast-reciprocal custom op (input must be at
    partition offset 0 - it silently corrupts otherwise), broadcast across
    partitions on GpSimd.
  - All 8 x-chunks are prefetched into SBUF up front so no input DMA
    remains to contend with the collective transfers mid-body.
  - The token re-shard is split into TWO AllToAlls: blocks 0-5 fired
    mid-body (fully overlapped), and one small (256KB) tail collective
    for blocks 6-7, overlapped by the earlier blocks' out projections
    (a single tail collective avoids serializing twice on the ~10-25us
    per-op cc-core software cost). All output writes are deferred past
    the tail collective. Each core ends with 384+128 tokens (full
    1024-dim) and runs the out projection for those rows; the host
    stitches the disjoint row-blocks.
"""

import os
import sys
from contextlib import ExitStack

if "/opt/trn_rl_repo" not in sys.path:
    sys.path.insert(0, "/opt/trn_rl_repo")

import numpy as np

import concourse.mybir as mybir
import concourse.tile as tile
from concourse import bacc, bass_utils

B, S, D, H, HD = 2, 2048, 1024, 16, 64
NCORES = 8
SL = B * S            # 4096 tokens total
QC = 512              # q-chunk / moving free dim
KC = 128              # k-chunk (partition dim)
NQ = S // QC          # 4 q-chunks per batch
NK = S // KC          # 16 k-chunks per batch
NT = SL // QC         # 8 token chunks
DK = D // 128         # 8 contraction chunks of the model dim
VW = 2 * (HD + 1)     # 130: V-natural block width (2 heads x (64 V + ones col))
NA = int(os.environ.get("SMSA_NA", "6"))  # blocks in the first AllToAll
SA = NA * QC // NCORES         # a2a A slot width (384)
SB = (NT - NA) * QC // NCORES  # a2a B slot width (128: last two blocks)

f32 = mybir.dt.float32
bf16 = mybir.dt.bfloat16
FX = mybir.ActivationFunctionType
ALU = mybir.AluOpType

LAST_EXEC_NS = None   # HW exec time (ns) of the last traced run
LAST_RESULTS = None


def _build(variant, exp_bias=0.0):
    """Emit the SPMD program. variant: 'causal' | 'dense' | 'general'."""
    assert variant in ("causal", "dense", "general")
    nc = bacc.Bacc("TRN2", target_bir_lowering=False, debug=False,
                   num_devices=NCORES)

    xT_d = nc.dram_tensor("xT", [D, SL], bf16, kind="ExternalInput")
    wqkv_d = nc.dram_tensor("wqkv", [D, 384], bf16, kind="ExternalInput")
    bqkv_d = nc.dram_tensor("bqkv", [128, 3], f32, kind="ExternalInput")
    wout_d = nc.dram_tensor("wout", [D, D], bf16, kind="ExternalInput")
    bout_d = nc.dram_tensor("bout", [1, D], bf16, kind="ExternalInput")
    ident_d = nc.dram_tensor("ident", [128, 128], bf16, kind="ExternalInput")
    if variant == "causal":
        tri_d = nc.dram_tensor("tri", [128, 128], bf16, kind="ExternalInput")
    if variant == "general":
        maskT_d = nc.dram_tensor("maskT", [B, S, S], f32, kind="ExternalInput")
    out_d = nc.dram_tensor("out", [QC, D], f32, kind="ExternalOutput")

    with tile.TileContext(nc) as tc:
        with ExitStack() as stack:
            ep = stack.enter_context
            cpool = ep(tc.tile_pool(name="consts", bufs=1))
            big = ep(tc.tile_pool(name="big", bufs=1))
            xpool = ep(tc.tile_pool(name="xts", bufs=NT))
            vpool = ep(tc.tile_pool(name="vstg", bufs=2))
            epool = ep(tc.tile_pool(name="epool", bufs=12))
            mpool = ep(tc.tile_pool(name="mpool", bufs=4))
            rpool = ep(tc.tile_pool(name="rpool", bufs=2))
            apool = ep(tc.tile_pool(name="apool", bufs=2))
            ppool = ep(tc.tile_pool(name="ppool", bufs=2))
            opool = ep(tc.tile_pool(name="opool", bufs=2))
            dram = ep(tc.tile_pool(name="dram", bufs=1, space="DRAM"))
            psmm = ep(tc.tile_pool(name="psmm", bufs=2, space="PSUM"))
            pssc = ep(tc.tile_pool(name="pssc", bufs=4, space="PSUM"))
            psav0 = ep(tc.tile_pool(name="psav0", bufs=1, space="PSUM"))
            psav1 = ep(tc.tile_pool(name="psav1", bufs=1, space="PSUM"))

            # ---------------- constants / resident tensors ----------------
            # first-matmul-critical loads split 4 ways across the DMA rings
            w_sb = big.tile([128, DK * 384], bf16, name="w_sb")
            # load the m=0 (query) columns first: the very first PSUM group
            # only needs them + xt(0), shrinking the startup-critical bytes
            w_dst = w_sb[:].rearrange("p (a c) -> p a c", a=DK)
            w_src = wqkv_d.ap().rearrange("(a p) c -> p a c", p=128)
            for u in range(2):
                nc.sync.dma_start(w_dst[:, DK // 2 * u:DK // 2 * (u + 1), 0:128],
                                  w_src[:, DK // 2 * u:DK // 2 * (u + 1), 0:128])
            for u in range(2):
                nc.sync.dma_start(
                    w_dst[:, DK // 2 * u:DK // 2 * (u + 1), 128:384],
                    w_src[:, DK // 2 * u:DK // 2 * (u + 1), 128:384])
            bq_sb = cpool.tile([128, 3], f32, name="bq_sb")
            nc.sync.dma_start(bq_sb[:], bqkv_d.ap())
            ident = cpool.tile([128, 128], bf16, name="ident")
            nc.sync.dma_start(ident[:], ident_d.ap())
            if variant == "causal":
                tri = cpool.tile([128, 128], bf16, name="tri")
                nc.sync.dma_start(tri[:], tri_d.ap())

            ones_sb = cpool.tile([1, QC], bf16, name="ones_sb")
            nc.vector.memset(ones_sb[:], 1.0)

            qT = big.tile([128, SL], bf16, name="qT")
            kT = big.tile([128, SL], bf16, name="kT")
            vn = big.tile([128, B * NK * VW], bf16, name="vn")
            # ones columns for the softmax denominator live at 64 + 65*j
            vn_ones = vn[:].rearrange("p (b c) -> p b c", c=HD + 1)[:, :, 64:65]
            nc.vector.memset(vn_ones, 1.0)

            wo_sb = big.tile([128, DK * D], bf16, name="wo_sb")
            bo_sb = cpool.tile([1, D], bf16, name="bo_sb")
            bo_bc = big.tile([128, D], bf16, name="bo_bc")

            a2aA_in = dram.tile([NCORES, 128, SA], bf16, name="a2aA_in")
            a2aA_out = dram.tile([NCORES, 128, SA], bf16, name="a2aA_out")
            a2aB_in = dram.tile([NCORES, 128, SB], bf16, name="a2aB_in")
            a2aB_out = dram.tile([NCORES, 128, SB], bf16, name="a2aB_out")

            # ---------------- phase 1: QKV projection -----------------------
            # all x chunks are prefetched up front: the DMA rings drain them
            # during the early body, so no input loads remain to contend with
            # the mid-body collective transfer
            def load_xt(t, nsplit=2):
                xt = xpool.tile([128, DK * QC], bf16, name=f"xt{t}", tag="xt")
                hw_ = DK // nsplit * QC
                rw = D // nsplit
                for u in range(nsplit):   # parallel DMA rings
                    nc.sync.dma_start(
                        xt[:, hw_ * u:hw_ * (u + 1)]
                            .rearrange("p (a c) -> p a c", a=DK // nsplit),
                        xT_d.ap()[rw * u:rw * (u + 1), QC * t:QC * (t + 1)]
                            .rearrange("(a p) c -> p a c", p=128))
                return xt

            def emit_qkv(t, xt):
                for m in range(3):
                    ps = psmm.tile([128, QC], f32, name=f"qkv{t}_{m}", tag="mm")
                    for dk in range(DK):
                        c0 = 384 * dk + 128 * m
                        nc.tensor.matmul(ps[:],
                                         w_sb[:, c0:c0 + 128],
                                         xt[:, QC * dk:QC * (dk + 1)],
                                         start=(dk == 0), stop=(dk == DK - 1))
                    bias_ap = bq_sb[:, m:m + 1]
                    if m == 0:
                        nc.vector.tensor_scalar_add(
                            out=qT[:, QC * t:QC * (t + 1)], in0=ps[:], scalar1=bias_ap)
                    elif m == 1:
                        nc.vector.tensor_scalar_add(
                            out=kT[:, QC * t:QC * (t + 1)], in0=ps[:], scalar1=bias_ap)
                    else:
                        vst = vpool.tile([128, QC], bf16, name=f"vst{t}", tag="vst")
                        nc.vector.tensor_scalar_add(out=vst[:], in0=ps[:], scalar1=bias_ap)
                        for ci in range(4):
                            gi = 4 * t + ci
                            # share the psmm ring (2KB slots hold the 256B tile)
                            trp = psmm.tile([128, 128], bf16, name=f"tr{gi}", tag="mm")
                            nc.tensor.transpose(trp[:], vst[:, 128 * ci:128 * (ci + 1)],
                                                ident[:])
                            # both heads' V columns in one strided copy
                            # (skipping the ones column at +64 of each group)
                            nc.vector.tensor_copy(
                                out=vn[:].rearrange("p (g x) -> p g x", x=HD + 1)
                                    [:, 2 * gi:2 * gi + 2, 0:HD],
                                in_=trp[:].rearrange("p (a c) -> p a c", a=2))

            # ---------------- phase 2: attention for one (b, j) block ------
            def emit_attn(b, j):
                n_i = 4 * (j + 1) if variant == "causal" else NK
                q0 = S * b + QC * j
                av0 = psav0.tile([65, QC], f32, name=f"av0_{b}_{j}", tag="av0")
                av1 = psav1.tile([65, QC], f32, name=f"av1_{b}_{j}", tag="av1")

                def emit_av(e0, e1, gi, i, w0):
                    st, sp_ = (i == 0), (i == n_i - 1)
                    nc.tensor.matmul(av0[:, w0:],
                                     vn[:, VW * gi:VW * gi + 65],
                                     e0[:, w0:], start=st, stop=sp_,
                                     skip_group_check=True)
                    nc.tensor.matmul(av1[:, w0:],
                                     vn[:, VW * gi + 65:VW * gi + 130],
                                     e1[:, w0:], start=st, stop=sp_,
                                     skip_group_check=True)

                pend = []
                for i in range(n_i):
                    gi = NK * b + i
                    k0 = S * b + KC * i
                    diag = variant == "causal" and i >= n_i - 4
                    m = i - 4 * j if diag else 0   # diagonal offset 0..3
                    w0 = 128 * m                   # live query window start
                    s0 = pssc.tile([128, QC], f32, name=f"s0_{b}_{j}_{i}", tag="sc")
                    s1 = pssc.tile([128, QC], f32, name=f"s1_{b}_{j}_{i}", tag="sc")
                    nc.tensor.matmul(s0[:, w0:], kT[0:64, k0:k0 + KC],
                                     qT[0:64, q0 + w0:q0 + QC],
                                     start=True, stop=True)
                    nc.tensor.matmul(s1[:, w0:], kT[64:128, k0:k0 + KC],
                                     qT[64:128, q0 + w0:q0 + QC],
                                     start=True, stop=True)
                    if variant == "general":
                        mt = mpool.tile([128, QC], f32, name=f"mt{b}_{j}_{i}", tag="mt")
                        nc.sync.dma_start(
                            mt[:], maskT_d.ap()[b, KC * i:KC * (i + 1),
                                                QC * j:QC * (j + 1)])
                        nc.vector.tensor_tensor(out=s0[:], in0=s0[:], in1=mt[:],
                                                op=ALU.add)
                        nc.vector.tensor_tensor(out=s1[:], in0=s1[:], in1=mt[:],
                                                op=ALU.add)
                    e0 = epool.tile([128, QC], bf16, name=f"e0_{b}_{j}_{i}", tag="e")
                    e1 = epool.tile([128, QC], bf16, name=f"e1_{b}_{j}_{i}", tag="e")
                    nc.scalar.activation(out=e0[:, w0:], in_=s0[:, w0:], func=FX.Exp,
                                         bias=exp_bias)
                    nc.scalar.activation(out=e1[:, w0:], in_=s1[:, w0:], func=FX.Exp,
                                         bias=exp_bias)
                    if diag:
                        # zero exp(score) above the diagonal inside the
                        # partial 128-col sub-block: keep col c of partition
                        # p iff c >= p (cols beyond the sub-block all pass)
                        for e in (e0, e1):
                            nc.vector.tensor_tensor(
                                out=e[:, w0:w0 + 128], in0=e[:, w0:w0 + 128],
                                in1=tri[:], op=ALU.mult)
                    pend.append((e0, e1, gi, i, w0))
                    # drain eagerly on the final block so its AV accumulation
                    # (and the tail collective behind it) finishes sooner
                    depth = 1 if (b, j) == (B - 1, NQ - 1) else 3
                    if len(pend) > depth:
                        emit_av(*pend.pop(0))
                while pend:
                    emit_av(*pend.pop(0))

                # softmax normalization + store the a2a quarter-chunks
                # (reciprocal_approx_fast requires partition-offset-0 input:
                #  stage the denominator row through a fresh [1, QC] tile)
                d0 = rpool.tile([1, QC], f32, name=f"d0_{b}_{j}", tag="d0")
                d1 = rpool.tile([1, QC], f32, name=f"d1_{b}_{j}", tag="d1")
                if (b, j) == (B - 1, NQ - 1):
                    # final block: ScalarE is idle by now; splitting the two
                    # denominator copies across engines shortens the critical
                    # path into the tail collective
                    nc.scalar.activation(out=d0[:], in_=av0[64:65, :],
                                         func=FX.Copy)
                else:
                    nc.vector.tensor_copy(out=d0[:], in_=av0[64:65, :])
                nc.vector.tensor_copy(out=d1[:], in_=av1[64:65, :])
                rr = rpool.tile([1, QC], f32, name=f"rr_{b}_{j}", tag="rr")
                rrb = rpool.tile([1, QC], bf16, name=f"rrb_{b}_{j}", tag="rrb")
                nc.vector.reciprocal_approx_fast(out=rr[:], in_=d0[:])
                nc.vector.tensor_copy(out=rrb[:], in_=rr[:])
                rr1 = rpool.tile([1, QC], f32, name=f"rr1_{b}_{j}", tag="rr1")
                rrb1 = rpool.tile([1, QC], bf16, name=f"rrb1_{b}_{j}", tag="rrb1")
                nc.vector.reciprocal_approx_fast(out=rr1[:], in_=d1[:])
                nc.vector.tensor_copy(out=rrb1[:], in_=rr1[:])
                bc0 = psmm.tile([128, QC], f32, name=f"bc0_{b}_{j}", tag="mm")
                nc.tensor.matmul(bc0[:], ones_sb[0:1, 0:128], rrb[:],
                                 start=True, stop=True)
                bc1 = psmm.tile([128, QC], f32, name=f"bc1_{b}_{j}", tag="mm")
                nc.tensor.matmul(bc1[:], ones_sb[0:1, 0:128], rrb1[:],
                                 start=True, stop=True)
                bs = apool.tile([128, QC], bf16, name=f"bs{b}_{j}", tag="bs")
                if (b, j) == (B - 1, NQ - 1):
                    # final block: ScalarE is idle; splitting the copies
                    # across engines shortens the serial finalize chain
                    # that gates the tail collective trigger
                    nc.scalar.activation(out=bs[0:64, :], in_=bc0[0:64, :],
                                         func=FX.Copy)
                else:
                    nc.vector.tensor_copy(out=bs[0:64, :], in_=bc0[0:64, :])
                nc.vector.tensor_copy(out=bs[64:128, :], in_=bc1[64:128, :])
                att = apool.tile([128, QC], bf16, name=f"att{b}_{j}", tag="att")
                nc.vector.tensor_tensor(out=att[0:64, :], in0=av0[0:64, :],
                                        in1=bs[0:64, :], op=ALU.mult)
                nc.vector.tensor_tensor(out=att[64:128, :], in0=av1[0:64, :],
                                        in1=bs[64:128, :], op=ALU.mult)
                bi = NQ * b + j
                if bi < NA:
                    # split the block's 512 cols at the slot boundaries
                    c0 = QC * bi
                    while c0 < QC * bi + QC:
                        n = c0 // SA
                        c1 = min(SA * (n + 1), QC * bi + QC)
                        nc.sync.dma_start(
                            a2aA_in[n][:, c0 - SA * n:c1 - SA * n],
                            att[:, c0 - QC * bi:c1 - QC * bi])
                        c0 = c1
                else:
                    # blocks NA..NT-1 feed a2aB: block h spans slots 4h..4h+3
                    h = bi - NA
                    nc.sync.dma_start(
                        a2aB_in[4 * h:4 * h + 4].rearrange("a p c -> p a c"),
                        att[:].rearrange("p (a c) -> p a c", a=4))

            # ---------------- phase 3: AllToAll + out projection -----------
            def emit_collective(in_t, out_t):
                nc.gpsimd.collective_compute(
                    "AllToAll", ALU.bypass,
                    replica_groups=[list(range(NCORES))],
                    ins=[in_t.opt()], outs=[out_t.opt()])

            def emit_outproj(out_t, width, row0, defer=None):
                at = ppool.tile([128, DK * width], bf16,
                                name=f"at{row0}", tag=f"at{row0}")
                for u in range(2):   # two DMAs -> two queues in parallel
                    nc.sync.dma_start(
                        at[:, DK // 2 * width * u:DK // 2 * width * (u + 1)]
                            .rearrange("p (a c) -> p a c", a=DK // 2),
                        out_t[DK // 2 * u:DK // 2 * (u + 1)]
                            .rearrange("a p c -> p a c"))
                for q0 in range(0, width, 128):
                    qw = min(128, width - q0)
                    osb = opool.tile([128, D], f32, bufs=4,
                                     name=f"osb{row0}_{q0}", tag="osb")
                    for dc in range(2):
                        ps = psmm.tile([128, QC], f32,
                                       name=f"op{row0}_{q0}_{dc}", tag="mm")
                        for dk in range(DK):
                            c0 = D * dk + QC * dc
                            a0 = width * dk + q0
                            nc.tensor.matmul(ps[0:qw, :], at[:, a0:a0 + qw],
                                             wo_sb[:, c0:c0 + QC],
                                             start=(dk == 0),
                                             stop=(dk == DK - 1))
                        nc.vector.tensor_tensor(
                            out=osb[0:qw, QC * dc:QC * (dc + 1)], in0=ps[0:qw, :],
                            in1=bo_bc[0:qw, QC * dc:QC * (dc + 1)], op=ALU.add)
                    r0 = row0 + q0
                    if defer is None:
                        nc.sync.dma_start(out_d.ap()[r0:r0 + qw, :], osb[0:qw, :])
                    else:
                        defer.append((osb, r0, qw))

            # ----- interleave qkv t-chunks, attention, collectives ---------
            blocks = [(b, j) for b in range(B) for j in range(NQ)]
            xts = [load_xt(t, nsplit=4 if t == 0 else 2) for t in range(NT)]
            for t in range(NT):
                emit_qkv(t, xts[t])
                if t == 2:
                    # out-projection weights, loaded during the body
                    nc.sync.dma_start(
                        wo_sb[:].rearrange("p (a c) -> p a c", a=DK),
                        wout_d.ap().rearrange("(a p) c -> p a c", p=128))
                    nc.sync.dma_start(bo_sb[:], bout_d.ap())
                    # pre-broadcast the output bias across partitions so the
                    # out projections fuse it into their PSUM->SBUF copy
                    # instead of a trailing rank-1 matmul each
                    for dc in range(2):
                        bcp = psmm.tile([128, QC], f32, name=f"bobc{dc}",
                                        tag="mm")
                        nc.tensor.matmul(bcp[:], ones_sb[0:1, 0:128],
                                         bo_sb[0:1, QC * dc:QC * (dc + 1)],
                                         start=True, stop=True)
                        nc.vector.tensor_copy(
                            out=bo_bc[:, QC * dc:QC * (dc + 1)], in_=bcp[:])
                if t >= 1:
                    emit_attn(*blocks[t - 1])
                if t == NA:
                    # blocks 0..NA-1 done; re-shard them while the remaining
                    # attention blocks compute (all input loads long drained)
                    emit_collective(a2aA_in, a2aA_out)
            emit_attn(*blocks[NT - 1])
            # out-proj A only needs collective A (long done): it runs on the
            # tensor queue right after the last attention block, overlapping
            # the tail collective B transfer. All output writes are deferred
            # past collective B so its transfer window sees no DMA traffic.
            emit_collective(a2aB_in, a2aB_out)
            deferred = []
            emit_outproj(a2aA_out, SA, 0, defer=deferred)
            emit_outproj(a2aB_out, SB, SA, defer=deferred)
            for osb, r0, qw in deferred:
                nc.sync.dma_start(out_d.ap()[r0:r0 + qw, :], osb[0:qw, :])

    nc.finalize()
    return nc


def _detect_variant(mask):
    if not mask.any():
        return "dense"
    tri = np.where(np.tril(np.ones((S, S), dtype=bool)),
                   np.float32(0.0), np.float32(-1e9)).astype(np.float32)
    for b in range(B):
        if not np.array_equal(mask[b], tri):
            return "general"
    return "causal"


def kernel(**inputs):
    global LAST_EXEC_NS, LAST_RESULTS
    import ml_dtypes
    bfl = ml_dtypes.bfloat16

    x = np.ascontiguousarray(np.asarray(inputs["x"], dtype=np.float32))
    mask = np.asarray(inputs["mask"], dtype=np.float32)
    w_qkv = np.asarray(inputs["w_qkv"], dtype=np.float32)
    b_qkv = np.asarray(inputs["b_qkv"], dtype=np.float32)
    w_out = np.ascontiguousarray(np.asarray(inputs["w_out"], dtype=np.float32))
    b_out = np.asarray(inputs["b_out"], dtype=np.float32)

    variant = _detect_variant(mask)

    maskT = None
    if variant in ("general", "dense"):
        # guard exp() against overflow: bound max score via norms; any
        # needed shift is folded into the (transposed) additive mask.
        xf = x.reshape(SL, D)
        qkv = xf @ w_qkv + b_qkv
        qkv = qkv.reshape(SL, H, 3 * HD)
        qn = np.linalg.norm(qkv[:, :, :HD], axis=2).max()
        kn = np.linalg.norm(qkv[:, :, HD:2 * HD], axis=2).max()
        mmax = 0.0 if variant == "dense" else max(0.0, float(np.nanmax(mask)))
        bound = qn * kn / np.sqrt(HD) + mmax
        shift = min(0.0, 60.0 - bound)
        if variant == "dense" and shift < 0.0:
            variant = "general"
        if variant == "general":
            maskT = np.ascontiguousarray(
                mask.transpose(0, 2, 1) + np.float32(shift))

    xT = np.ascontiguousarray(x.reshape(SL, D).T).astype(bfl)
    const_ident = np.eye(128, dtype=bfl)
    # keep e[p, c] iff c >= p within a diagonal 128-col sub-block
    const_tri = np.triu(np.ones((128, 128), dtype=np.float32)).astype(bfl)
    w_out_c = w_out.astype(bfl)
    bo = np.ascontiguousarray(b_out.reshape(1, D)).astype(bfl)

    in_maps = []
    for c in range(NCORES):
        h0, h1 = 2 * c, 2 * c + 1

        def wcol(h, o):
            return w_qkv[:, 192 * h + o:192 * h + o + 64]

        def bcol(h, o):
            return b_qkv[192 * h + o:192 * h + o + 64]

        wq = np.concatenate([wcol(h0, 0), wcol(h1, 0)], axis=1) * np.float32(0.125)
        wk = np.concatenate([wcol(h0, 64), wcol(h1, 64)], axis=1)
        wv = np.concatenate([wcol(h0, 128), wcol(h1, 128)], axis=1)
        wc = np.ascontiguousarray(
            np.concatenate([wq, wk, wv], axis=1)).astype(bfl)
        bq = np.concatenate([bcol(h0, 0), bcol(h1, 0)]) * np.float32(0.125)
        bk = np.concatenate([bcol(h0, 64), bcol(h1, 64)])
        bv = np.concatenate([bcol(h0, 128), bcol(h1, 128)])
        bc = np.ascontiguousarray(
            np.stack([bq, bk, bv], axis=1))  # (128, 3)

        m = {"xT": xT, "wqkv": wc, "bqkv": bc, "wout": w_out_c, "bout": bo,
             "ident": const_ident}
        if variant == "causal":
            m["tri"] = const_tri
        if variant == "general":
            m["maskT"] = maskT
        in_maps.append(m)

    nc = _build(variant)
    trace = os.environ.get("SMSA_TRACE", "0") == "1"
    res = bass_utils.run_bass_kernel_spmd(
        nc, in_maps, core_ids=list(range(NCORES)), trace=trace)
    LAST_EXEC_NS = res.exec_time_ns
    LAST_RESULTS = res

    # core c's out rows [0, SA) = flat tokens [SA*c, +SA); rows
    # [SA, SA+SB) = flat [NA*QC + SB*c, +SB) (blocks NA..NT-1)
    flat = np.empty((SL, D), dtype=np.float32)
    for c in range(NCORES):
        part = res.results[c]["out"]
        flat[SA * c:SA * (c + 1)] = part[0:SA]
        flat[NA * QC + SB * c:NA * QC + SB * (c + 1)] = part[SA:SA + SB]
    return np.ascontiguousarray(flat.reshape(B, S, D))
